# revision 3
# baseline (speedup 1.0000x reference)
"""BERT encoder (12 layers, B=16, S=512, D=768) on 8 Trainium2 NeuronCores.

Strategy: data-parallel over batch (2 sequences per core), no collectives.
On-device layout is feature-major: activations live in SBUF as [128, 6, 1024]
tiles holding X^T (feature f = ko*128 + p, token t = seq*512 + s).

Weights are converted to bf16 on the host (halves HBM traffic; activations
stay fp32r so matmul accumulation is fp32 in PSUM). All matmuls run with
fp32r moving operands (full-rate on the PE at free-dim >= 256).

Per-layer pipeline on each core:
  Q/K projections -> feature-major, bias fused into ACT eviction
  V projection    -> token-major into V_ext with an appended ones column
                     (PV matmul then yields both P@V and Z=sum_k P in one go)
  scores^T = K_h Q_h^T per (seq, head); exp(scale*s + mask) fused in one
  ACT pass (mask is a per-partition bias; no max-subtraction needed at these
  magnitudes); PV + normalization by replicated 1/Z; Wo with residual add;
  LayerNorm via ones-matmul column sums + K=1 replicate matmuls; fused FFN
  (H tiles never hit DRAM; W2 accumulation in PSUM).

The 12-layer stack runs as a hardware loop (tc.For_i) with layer-indexed
weight DMAs, wrapped in an outer rep loop so the program size is independent
of n_reps (the timing diff then measures pure HW execution).

Embedding gather/positional add and the final unshard run on the host.
"""

import sys
import os

sys.path.insert(0, "/opt/trn_rl_repo")

import numpy as np

import concourse.bass as bass
import concourse.mybir as mybir
import concourse.tile as tile
from concourse import bacc
from concourse.bass import ds
from concourse.bass_utils import run_bass_kernel_spmd

F32 = mybir.dt.float32
F32R = mybir.dt.float32r
BF16 = mybir.dt.bfloat16
AF = mybir.ActivationFunctionType
ALU = mybir.AluOpType

N_CORES = 8
V, D, H, L, FF, MAXLEN = 30522, 768, 12, 12, 3072, 1000
B, S = 16, 512
DH = D // H          # 64
KO = D // 128        # 6
FO = FF // 128       # 24
SB = B // N_CORES    # 2 sequences per core
T = SB * S           # 1024 tokens per core
SCALE = 1.0 / np.sqrt(DH)
NEG = -1.0e6
# bias pack columns: [bq 0:6 | bk 6:12 | bo_eff 12:18 | b2 18:24 | b1 24:48]
NBIAS = 48


def _build(n_reps=1):
    nc = bacc.Bacc("TRN2", target_bir_lowering=False, debug=False,
                   num_devices=N_CORES)

    x0_d = nc.dram_tensor("x0", [128, KO, T], F32, kind="ExternalInput").ap()
    msk_d = nc.dram_tensor("mask", [128, SB * 4], F32, kind="ExternalInput").ap()
    wq_d = nc.dram_tensor("WQ", [L, D, D], BF16, kind="ExternalInput").ap()
    wk_d = nc.dram_tensor("WK", [L, D, D], BF16, kind="ExternalInput").ap()
    wv_d = nc.dram_tensor("WV", [L, D, D], BF16, kind="ExternalInput").ap()
    wo_d = nc.dram_tensor("WO", [L, D, D], BF16, kind="ExternalInput").ap()
    w1_d = nc.dram_tensor("W1", [L, D, FF], BF16, kind="ExternalInput").ap()
    w2_d = nc.dram_tensor("W2", [L, FF, D], BF16, kind="ExternalInput").ap()
    bias_d = nc.dram_tensor("BIAS", [L, 128, NBIAS], F32, kind="ExternalInput").ap()
    xout_d = nc.dram_tensor("xout", [128, KO, T], F32, kind="ExternalOutput").ap()

    # layer-indexed rearranged views for dynamic (For_i) weight DMAs
    wq_r = wq_d.rearrange("l (ko p) m -> l p ko m", p=128)
    wk_r = wk_d.rearrange("l (ko p) m -> l p ko m", p=128)
    wv_r = wv_d.rearrange("l (ko p) m -> l p ko m", p=128)
    wo_r = wo_d.rearrange("l (ko p) m -> l p ko m", p=128)
    w1_r = w1_d.rearrange("l (ko p) m -> l p ko m", p=128)

    with tile.TileContext(nc) as tc:
        from contextlib import ExitStack
        with nc.allow_low_precision(reason="bf16 weights, fp32r accumulate"), \
                ExitStack() as ctx:
            persist = ctx.enter_context(tc.tile_pool(name="persist", bufs=1))
            biasp = ctx.enter_context(tc.tile_pool(name="biasp", bufs=2))
            wpool = ctx.enter_context(tc.tile_pool(name="wpool", bufs=2))
            w1p = ctx.enter_context(tc.tile_pool(name="w1p", bufs=2))
            w2p = ctx.enter_context(tc.tile_pool(name="w2p", bufs=2))
            hpool = ctx.enter_context(tc.tile_pool(name="hpool", bufs=3))
            probs = ctx.enter_context(tc.tile_pool(name="probs", bufs=2))
            rzp = ctx.enter_context(tc.tile_pool(name="rzp", bufs=1))
            ps = ctx.enter_context(tc.tile_pool(name="ps", bufs=8, space="PSUM"))

            X = persist.tile([128, KO, T], F32R)
            R = persist.tile([128, KO, T], F32R)
            Q = persist.tile([128, KO, T], F32R)
            K = persist.tile([128, KO, T], F32R)
            SQ = persist.tile([128, KO, T], BF16)
            VE = persist.tile([128, SB * 4, H * 65], F32R)
            msk = persist.tile([128, SB * 4], F32)
            mean_t = persist.tile([1, T], F32)
            msq_t = persist.tile([1, T], F32)
            a_t = persist.tile([1, T], F32R)
            b_t = persist.tile([1, T], F32R)

            # constants
            of1 = persist.tile([128, 1], F32)
            nc.vector.memset(of1[:], 1.0)
            ones128r = persist.tile([128, 1], F32R)
            nc.vector.tensor_copy(ones128r[:], of1[:])
            ones128b = persist.tile([128, 1], BF16)
            nc.vector.tensor_copy(ones128b[:], of1[:])
            of2 = persist.tile([1, 128], F32)
            nc.vector.memset(of2[:], 1.0)
            ones1r = persist.tile([1, 128], F32R)
            nc.vector.tensor_copy(ones1r[:], of2[:])
            eps_t = persist.tile([1, 1], F32)
            nc.vector.memset(eps_t[:], 1e-5)
            of96 = persist.tile([128, SB * 4, H], F32)
            nc.vector.memset(of96[:], 1.0)
            nc.vector.tensor_copy(VE[:, :, 64::65], of96[:])

            nc.sync.dma_start(msk[:], msk_d[:])
            nc.sync.dma_start(X[:], x0_d[:].bitcast(F32R))

            def psum(p_dim, f_dim):
                return ps.tile([p_dim, f_dim], F32, tag="ps", name="ps")

            def qk_proj(li, w_view, bcol, dst, bias_sb):
                wts = []
                for half in range(2):
                    wt = wpool.tile([128, 3, D], BF16, tag="w")
                    nc.sync.dma_start(
                        wt[:], w_view[ds(li, 1), :, 3 * half:3 * half + 3, :])
                    wts.append(wt)
                for m in range(KO):
                    pss = (psum(128, 512), psum(128, 512))
                    for kt in range(KO):
                        lhsT = wts[kt // 3][:, kt % 3, m * 128:(m + 1) * 128]
                        for c in range(2):
                            nc.tensor.matmul(
                                pss[c][:], lhsT, X[:, kt, c * 512:(c + 1) * 512],
                                start=(kt == 0), stop=(kt == KO - 1))
                    for c in range(2):
                        nc.scalar.activation(
                            dst[:, m, c * 512:(c + 1) * 512], pss[c][:],
                            AF.Identity, bias=bias_sb[:, bcol + m:bcol + m + 1])
                return wts

            def v_proj(li, bias_sb):
                wts = []
                for half in range(2):
                    wt = wpool.tile([128, 3, D], BF16, tag="w")
                    nc.sync.dma_start(
                        wt[:], wv_r[ds(li, 1), :, 3 * half:3 * half + 3, :])
                    wts.append(wt)
                for tt in range(SB * 4):
                    for c2 in range(2):
                        pv2 = psum(128, 384)
                        for kt in range(KO):
                            nc.tensor.matmul(
                                pv2[:], X[:, kt, tt * 128:(tt + 1) * 128],
                                wts[kt // 3][:, kt % 3, c2 * 384:(c2 + 1) * 384],
                                start=(kt == 0), stop=(kt == KO - 1))
                        dst = VE[:, tt, c2 * 390:c2 * 390 + 390]
                        dst = dst.rearrange("p (h c) -> p h c", c=65)[:, :, 0:64]
                        nc.scalar.activation(
                            dst, pv2[:].rearrange("p (h c) -> p h c", c=64),
                            AF.Copy)

            def attention():
                for s in range(SB):
                    for h in range(H):
                        p0 = (h % 2) * 64
                        ko = h // 2
                        q_ap = Q[p0:p0 + 64, ko, s * 512:(s + 1) * 512]
                        P_sb = probs.tile([128, 4, 512], F32R, tag="p")
                        for kt in range(4):
                            k_ap = K[p0:p0 + 64, ko,
                                     s * 512 + kt * 128:s * 512 + (kt + 1) * 128]
                            sc = psum(128, 512)
                            nc.tensor.matmul(sc[:], k_ap, q_ap, start=True, stop=True)
                            nc.scalar.activation(
                                P_sb[:, kt, :], sc[:], AF.Exp,
                                bias=msk[:, s * 4 + kt:s * 4 + kt + 1], scale=SCALE)
                        pv = psum(65, 512)
                        for kt in range(4):
                            nc.tensor.matmul(
                                pv[:], VE[:, 4 * s + kt, h * 65:(h + 1) * 65],
                                P_sb[:, kt, :], start=(kt == 0), stop=(kt == 3))
                        rz = rzp.tile([1, 512], F32R, tag="rz")
                        nc.vector.reciprocal(rz[:], pv[64:65, :])
                        repz = psum(64, 512)
                        nc.tensor.matmul(repz[:], ones1r[0:1, 0:64], rz[:],
                                         start=True, stop=True)
                        o_ap = Q[p0:p0 + 64, ko, s * 512:(s + 1) * 512]
                        nc.scalar.activation(o_ap, pv[0:64, :], AF.Copy)
                        nc.vector.tensor_tensor(o_ap, o_ap, repz[:], ALU.mult)

            def out_proj(li, bias_sb):
                wts = []
                for half in range(2):
                    wt = wpool.tile([128, 3, D], BF16, tag="w")
                    nc.sync.dma_start(
                        wt[:], wo_r[ds(li, 1), :, 3 * half:3 * half + 3, :])
                    wts.append(wt)
                for m in range(KO):
                    pss = (psum(128, 512), psum(128, 512))
                    for kt in range(KO):
                        lhsT = wts[kt // 3][:, kt % 3, m * 128:(m + 1) * 128]
                        for c in range(2):
                            nc.tensor.matmul(
                                pss[c][:], lhsT, Q[:, kt, c * 512:(c + 1) * 512],
                                start=(kt == 0), stop=(kt == KO - 1))
                    for c in range(2):
                        cs = slice(c * 512, (c + 1) * 512)
                        nc.vector.tensor_tensor(R[:, m, cs], pss[c][:],
                                                X[:, m, cs], ALU.add)
                        nc.vector.tensor_scalar(
                            R[:, m, cs], R[:, m, cs],
                            bias_sb[:, 12 + m:13 + m], None, op0=ALU.add)

            def layer_norm():
                nc.scalar.activation(SQ[:], R[:], AF.Square)
                for c in range(2):
                    cs = slice(c * 512, (c + 1) * 512)
                    ps_s = psum(1, 512)
                    for kt in range(KO):
                        nc.tensor.matmul(ps_s[:], ones128r[:], R[:, kt, cs],
                                         start=(kt == 0), stop=(kt == KO - 1))
                    ps_q = psum(1, 512)
                    for kt in range(KO):
                        nc.tensor.matmul(ps_q[:], ones128b[:], SQ[:, kt, cs],
                                         start=(kt == 0), stop=(kt == KO - 1))
                    nc.vector.tensor_scalar_mul(mean_t[0:1, cs], ps_s[:], 1.0 / D)
                    nc.vector.tensor_scalar_mul(msq_t[0:1, cs], ps_q[:], 1.0 / D)
                nc.scalar.activation(b_t[:], mean_t[:], AF.Square)
                nc.vector.tensor_sub(msq_t[:], msq_t[:], b_t[:])
                nc.scalar.activation(b_t[:], msq_t[:], AF.Sqrt,
                                     bias=eps_t[:])
                nc.vector.reciprocal(a_t[:], b_t[:])
                nc.vector.tensor_mul(b_t[:], mean_t[:], a_t[:])
                for c in range(2):
                    cs = slice(c * 512, (c + 1) * 512)
                    pa = psum(128, 512)
                    nc.tensor.matmul(pa[:], ones1r[:], a_t[0:1, cs],
                                     start=True, stop=True)
                    pb = psum(128, 512)
                    nc.tensor.matmul(pb[:], ones1r[:], b_t[0:1, cs],
                                     start=True, stop=True)
                    pa_b = pa[:][:, None, :].to_broadcast((128, KO, 512))
                    pb_b = pb[:][:, None, :].to_broadcast((128, KO, 512))
                    nc.vector.tensor_tensor(X[:, :, cs], R[:, :, cs], pa_b,
                                            ALU.mult)
                    nc.vector.tensor_tensor(X[:, :, cs], X[:, :, cs], pb_b,
                                            ALU.subtract)

            def ffn(li, bias_sb):
                for qc in range(2):
                    qs = slice(qc * 512, (qc + 1) * 512)
                    ys = [psum(128, 512) for _ in range(KO)]
                    for ft in range(FO):
                        if ft % 2 == 0:
                            # 2-ft granularity keeps bf16 DRAM runs at 512B
                            w1t2 = w1p.tile([128, KO, 256], BF16, tag="w1")
                            nc.sync.dma_start(
                                w1t2[:],
                                w1_r[ds(li, 1), :, :, ft * 128:(ft + 2) * 128])
                        w1t = w1t2[:, :, (ft % 2) * 128:(ft % 2) * 128 + 128]
                        w2t = w2p.tile([128, D], BF16, tag="w2")
                        nc.sync.dma_start(
                            w2t[:], w2_d[ds(li, 1), ft * 128:(ft + 1) * 128, :])
                        ph = psum(128, 512)
                        for kt in range(KO):
                            nc.tensor.matmul(ph[:], w1t[:, kt, :], X[:, kt, qs],
                                             start=(kt == 0), stop=(kt == KO - 1))
                        h_sb = hpool.tile([128, 512], F32R, tag="h")
                        nc.scalar.activation(h_sb[:], ph[:], AF.Relu,
                                             bias=bias_sb[:, 24 + ft:25 + ft])
                        for j in range(KO):
                            nc.tensor.matmul(
                                ys[j][:], w2t[:, j * 128:(j + 1) * 128], h_sb[:],
                                start=(ft == 0), stop=(ft == FO - 1),
                                skip_group_check=True)
                    for j in range(KO):
                        nc.vector.tensor_tensor(R[:, j, qs], ys[j][:], X[:, j, qs],
                                                ALU.add)
                        nc.vector.tensor_scalar(
                            R[:, j, qs], R[:, j, qs],
                            bias_sb[:, 18 + j:19 + j], None, op0=ALU.add)

            with tc.For_i(0, n_reps) as _rep:
                with tc.For_i(0, L) as li:
                    bias_sb = biasp.tile([128, NBIAS], F32, tag="b")
                    nc.sync.dma_start(bias_sb[:], bias_d[ds(li, 1), :, :])
                    qk_proj(li, wq_r, 0, Q, bias_sb)
                    qk_proj(li, wk_r, 6, K, bias_sb)
                    v_proj(li, bias_sb)
                    attention()
                    out_proj(li, bias_sb)
                    layer_norm()
                    ffn(li, bias_sb)
                    layer_norm()

            nc.sync.dma_start(xout_d[:], X[:].bitcast(F32))

    nc.compile()
    return nc


_NC_CACHE = {}


def _get_nc(n_reps=1):
    if n_reps not in _NC_CACHE:
        _NC_CACHE[n_reps] = _build(n_reps)
    return _NC_CACHE[n_reps]


def _to_bf16(a):
    # round-to-nearest-even fp32 -> bf16, stored as the raw uint16 view
    # reinterpreted through ml_dtypes for numpy transport
    import ml_dtypes
    return np.asarray(a, dtype=np.float32).astype(ml_dtypes.bfloat16)


def _prep_in_maps(inputs):
    tokens = np.asarray(inputs["tokens"])
    segments = np.asarray(inputs["segments"])
    valid_lens = np.asarray(inputs["valid_lens"])
    tok_emb = np.asarray(inputs["tok_emb"], dtype=np.float32)
    seg_emb = np.asarray(inputs["seg_emb"], dtype=np.float32)
    pos_emb = np.asarray(inputs["pos_emb"], dtype=np.float32)
    Wq = np.ascontiguousarray(np.asarray(inputs["Wq"], dtype=np.float32))
    Wk = np.ascontiguousarray(np.asarray(inputs["Wk"], dtype=np.float32))
    Wv = np.ascontiguousarray(np.asarray(inputs["Wv"], dtype=np.float32))
    Wo = np.ascontiguousarray(np.asarray(inputs["Wo"], dtype=np.float32))
    W1 = np.ascontiguousarray(np.asarray(inputs["W1"], dtype=np.float32))
    W2 = np.ascontiguousarray(np.asarray(inputs["W2"], dtype=np.float32))

    # host-side embedding + positional/segment add
    X0 = tok_emb[tokens] + seg_emb[segments] + pos_emb[0, :S][None, :, :]
    X0 = X0.astype(np.float32)

    # bias pack: bq | bk | bo_eff = bv @ Wo + bo | b2 | b1
    bo_eff = np.einsum("ld,ldm->lm", np.asarray(inputs["bv"], np.float32), Wo) \
        + np.asarray(inputs["bo"], np.float32)
    bias_pack = np.empty((L, 128, NBIAS), np.float32)
    for name, lo in (("bq", 0), ("bk", 6)):
        arr = np.asarray(inputs[name], np.float32)
        bias_pack[:, :, lo:lo + 6] = arr.reshape(L, 6, 128).transpose(0, 2, 1)
    bias_pack[:, :, 12:18] = bo_eff.reshape(L, 6, 128).transpose(0, 2, 1)
    bias_pack[:, :, 18:24] = np.asarray(inputs["b2"], np.float32) \
        .reshape(L, 6, 128).transpose(0, 2, 1)
    bias_pack[:, :, 24:48] = np.asarray(inputs["b1"], np.float32) \
        .reshape(L, 24, 128).transpose(0, 2, 1)
    bias_pack = np.ascontiguousarray(bias_pack)

    shared = dict(WQ=_to_bf16(Wq), WK=_to_bf16(Wk), WV=_to_bf16(Wv),
                  WO=_to_bf16(Wo), W1=_to_bf16(W1), W2=_to_bf16(W2),
                  BIAS=bias_pack)

    kpos = np.arange(S)
    in_maps = []
    for c in range(N_CORES):
        xc = X0[SB * c:SB * (c + 1)].reshape(T, D).T            # [768, 1024]
        xc = np.ascontiguousarray(
            xc.reshape(KO, 128, T).transpose(1, 0, 2))          # [128, 6, 1024]
        mc = np.zeros((128, SB * 4), np.float32)
        for s in range(SB):
            vl = valid_lens[SB * c + s]
            m = np.where(kpos >= vl, np.float32(NEG), np.float32(0.0))
            mc[:, s * 4:(s + 1) * 4] = m.reshape(4, 128).T
        in_maps.append(dict(x0=xc, mask=np.ascontiguousarray(mc), **shared))
    return in_maps


def _unshard(results):
    out = np.empty((B, S, D), np.float32)
    for c in range(N_CORES):
        xt = results[c]["xout"]                                  # [128, 6, 1024]
        xt = xt.transpose(1, 0, 2).reshape(D, T)                 # [768, 1024]
        out[SB * c:SB * (c + 1)] = xt.T.reshape(SB, S, D)
    return out


def kernel(**inputs):
    nc = _get_nc(1)
    in_maps = _prep_in_maps(inputs)
    res = run_bass_kernel_spmd(nc, in_maps, list(range(N_CORES)), trace=False)
    return _unshard(res.results)


if __name__ == "__main__":
    import time
    t0 = time.time()
    nc = _build(int(os.environ.get("NREPS", "1")))
    print("build+compile time:", time.time() - t0)


# revision 7
# speedup vs baseline: 452.5685x; 452.5685x over previous
"""BERT encoder (12 layers, B=16, S=512, D=768) on 8 Trainium2 NeuronCores.

Strategy: data-parallel over batch (2 sequences per core), no collectives.
On-device layout is feature-major: activations live in SBUF as [128, 6, 1024]
tiles holding X^T (feature f = ko*128 + p, token t = seq*512 + s).

Weights are converted to bf16 on the host (halves HBM traffic), and all
matmul operands are bf16 (PSUM accumulation stays fp32). The fp32 master
copy of the activations (X) drives the residual adds and LayerNorm; a bf16
shadow (XB) is refreshed after each LayerNorm for use as matmul input.

Per-layer pipeline on each core:
  Q/K projections -> feature-major, bias fused into ACT eviction
  V projection    -> token-major into V_ext with an appended ones column
                     (PV matmul then yields both P@V and Z=sum_k P in one go)
  scores^T = K_h Q_h^T per (seq, head); exp(scale*s + mask) fused in one
  ACT pass (mask is a per-partition bias; no max-subtraction needed at these
  magnitudes); PV + normalization by replicated 1/Z; Wo with residual add;
  LayerNorm via ones-matmul column sums + K=1 replicate matmuls; fused FFN
  (H tiles never hit DRAM; W2 accumulation in PSUM).

The 12-layer stack runs as a hardware loop (tc.For_i) with layer-indexed
weight DMAs, wrapped in an outer rep loop so the program size is independent
of n_reps (the timing diff then measures pure HW execution).

Embedding gather/positional add and the final unshard run on the host.
"""

import sys
import os

sys.path.insert(0, "/opt/trn_rl_repo")

import numpy as np

import concourse.bass as bass
import concourse.mybir as mybir
import concourse.tile as tile
from concourse import bacc
from concourse.bass import ds
from concourse.bass_utils import run_bass_kernel_spmd

F32 = mybir.dt.float32
F32R = mybir.dt.float32r
BF16 = mybir.dt.bfloat16
AF = mybir.ActivationFunctionType
ALU = mybir.AluOpType

N_CORES = 8
V, D, H, L, FF, MAXLEN = 30522, 768, 12, 12, 3072, 1000
B, S = 16, 512
DH = D // H          # 64
KO = D // 128        # 6
FO = FF // 128       # 24
SB = B // N_CORES    # 2 sequences per core
T = SB * S           # 1024 tokens per core
SCALE = 1.0 / np.sqrt(DH)
NEG = -1.0e6
# bias pack columns: [bq 0:6 | bk 6:12 | bo_eff 12:18 | b2 18:24 | b1 24:48]
NBIAS = 48


def _build(n_reps=1):
    nc = bacc.Bacc("TRN2", target_bir_lowering=False, debug=False,
                   num_devices=N_CORES)

    x0_d = nc.dram_tensor("x0", [128, KO, T], F32, kind="ExternalInput").ap()
    msk_d = nc.dram_tensor("mask", [128, SB * 4], F32, kind="ExternalInput").ap()
    wq_d = nc.dram_tensor("WQ", [L, D, D], BF16, kind="ExternalInput").ap()
    wk_d = nc.dram_tensor("WK", [L, D, D], BF16, kind="ExternalInput").ap()
    wv_d = nc.dram_tensor("WV", [L, D, D], BF16, kind="ExternalInput").ap()
    wo_d = nc.dram_tensor("WO", [L, D, D], BF16, kind="ExternalInput").ap()
    w1_d = nc.dram_tensor("W1", [L, D, FF], BF16, kind="ExternalInput").ap()
    w2_d = nc.dram_tensor("W2", [L, FF, D], BF16, kind="ExternalInput").ap()
    bias_d = nc.dram_tensor("BIAS", [L, 128, NBIAS], F32, kind="ExternalInput").ap()
    xout_d = nc.dram_tensor("xout", [128, KO, T], F32, kind="ExternalOutput").ap()

    # layer-indexed rearranged views for dynamic (For_i) weight DMAs
    wq_r = wq_d.rearrange("l (ko p) m -> l p ko m", p=128)
    wk_r = wk_d.rearrange("l (ko p) m -> l p ko m", p=128)
    wv_r = wv_d.rearrange("l (ko p) m -> l p ko m", p=128)
    wo_r = wo_d.rearrange("l (ko p) m -> l p ko m", p=128)
    w1_r = w1_d.rearrange("l (ko p) m -> l p ko m", p=128)

    with tile.TileContext(nc) as tc:
        from contextlib import ExitStack
        with nc.allow_low_precision(reason="bf16 weights, fp32r accumulate"), \
                ExitStack() as ctx:
            persist = ctx.enter_context(tc.tile_pool(name="persist", bufs=1))
            biasp = ctx.enter_context(tc.tile_pool(name="biasp", bufs=2))
            wpool = ctx.enter_context(tc.tile_pool(name="wpool", bufs=2))
            w1p = ctx.enter_context(tc.tile_pool(name="w1p", bufs=2))
            w2p = ctx.enter_context(tc.tile_pool(name="w2p", bufs=2))
            hpool = ctx.enter_context(tc.tile_pool(name="hpool", bufs=3))
            probs = ctx.enter_context(tc.tile_pool(name="probs", bufs=2))
            rzp = ctx.enter_context(tc.tile_pool(name="rzp", bufs=1))
            ps = ctx.enter_context(tc.tile_pool(name="ps", bufs=8, space="PSUM"))

            X = persist.tile([128, KO, T], F32R)
            R = persist.tile([128, KO, T], F32R)
            Q = persist.tile([128, KO, T], BF16)
            K = persist.tile([128, KO, T], BF16)
            XB = persist.tile([128, KO, T], BF16)
            SQ = persist.tile([128, KO, T], BF16)
            VE = persist.tile([128, SB * 4, H * 65], BF16)
            msk = persist.tile([128, SB * 4], F32)
            mean_t = persist.tile([1, T], F32)
            msq_t = persist.tile([1, T], F32)
            a_t = persist.tile([1, T], F32R)
            b_t = persist.tile([1, T], F32R)

            # constants
            of1 = persist.tile([128, 1], F32)
            nc.vector.memset(of1[:], 1.0)
            ones128r = persist.tile([128, 1], F32R)
            nc.vector.tensor_copy(ones128r[:], of1[:])
            ones128b = persist.tile([128, 1], BF16)
            nc.vector.tensor_copy(ones128b[:], of1[:])
            of2 = persist.tile([1, 128], F32)
            nc.vector.memset(of2[:], 1.0)
            ones1r = persist.tile([1, 128], F32R)
            nc.vector.tensor_copy(ones1r[:], of2[:])
            ones1b = persist.tile([1, 128], BF16)
            nc.vector.tensor_copy(ones1b[:], of2[:])
            eps_t = persist.tile([1, 1], F32)
            nc.vector.memset(eps_t[:], 1e-5)
            of96 = persist.tile([128, SB * 4, H], F32)
            nc.vector.memset(of96[:], 1.0)
            nc.vector.tensor_copy(VE[:, :, 64::65], of96[:])

            nc.sync.dma_start(msk[:], msk_d[:])
            nc.sync.dma_start(X[:], x0_d[:].bitcast(F32R))
            nc.scalar.activation(XB[:], X[:], AF.Copy)

            def psum(p_dim, f_dim):
                return ps.tile([p_dim, f_dim], F32, tag="ps", name="ps")

            def qk_proj(li, w_view, bcol, dst, bias_sb):
                wts = []
                for half in range(2):
                    wt = wpool.tile([128, 3, D], BF16, tag="w")
                    nc.sync.dma_start(
                        wt[:], w_view[ds(li, 1), :, 3 * half:3 * half + 3, :])
                    wts.append(wt)
                for m in range(KO):
                    pss = (psum(128, 512), psum(128, 512))
                    for kt in range(KO):
                        lhsT = wts[kt // 3][:, kt % 3, m * 128:(m + 1) * 128]
                        for c in range(2):
                            nc.tensor.matmul(
                                pss[c][:], lhsT, XB[:, kt, c * 512:(c + 1) * 512],
                                start=(kt == 0), stop=(kt == KO - 1))
                    for c in range(2):
                        nc.scalar.activation(
                            dst[:, m, c * 512:(c + 1) * 512], pss[c][:],
                            AF.Identity, bias=bias_sb[:, bcol + m:bcol + m + 1])
                return wts

            def v_proj(li, bias_sb):
                wts = []
                for half in range(2):
                    wt = wpool.tile([128, 3, D], BF16, tag="w")
                    nc.sync.dma_start(
                        wt[:], wv_r[ds(li, 1), :, 3 * half:3 * half + 3, :])
                    wts.append(wt)
                for tt in range(SB * 4):
                    for c2 in range(2):
                        pv2 = psum(128, 384)
                        for kt in range(KO):
                            nc.tensor.matmul(
                                pv2[:], XB[:, kt, tt * 128:(tt + 1) * 128],
                                wts[kt // 3][:, kt % 3, c2 * 384:(c2 + 1) * 384],
                                start=(kt == 0), stop=(kt == KO - 1))
                        dst = VE[:, tt, c2 * 390:c2 * 390 + 390]
                        dst = dst.rearrange("p (h c) -> p h c", c=65)[:, :, 0:64]
                        nc.scalar.activation(
                            dst, pv2[:].rearrange("p (h c) -> p h c", c=64),
                            AF.Copy)

            def attention():
                for s in range(SB):
                    for h in range(H):
                        p0 = (h % 2) * 64
                        ko = h // 2
                        q_ap = Q[p0:p0 + 64, ko, s * 512:(s + 1) * 512]
                        P_sb = probs.tile([128, 4, 512], BF16, tag="p")
                        for kt in range(4):
                            k_ap = K[p0:p0 + 64, ko,
                                     s * 512 + kt * 128:s * 512 + (kt + 1) * 128]
                            sc = psum(128, 512)
                            nc.tensor.matmul(sc[:], k_ap, q_ap, start=True, stop=True)
                            nc.scalar.activation(
                                P_sb[:, kt, :], sc[:], AF.Exp,
                                bias=msk[:, s * 4 + kt:s * 4 + kt + 1], scale=SCALE)
                        pv = psum(65, 512)
                        for kt in range(4):
                            nc.tensor.matmul(
                                pv[:], VE[:, 4 * s + kt, h * 65:(h + 1) * 65],
                                P_sb[:, kt, :], start=(kt == 0), stop=(kt == 3))
                        rz = rzp.tile([1, 512], BF16, tag="rz")
                        nc.vector.reciprocal(rz[:], pv[64:65, :])
                        repz = psum(64, 512)
                        nc.tensor.matmul(repz[:], ones1b[0:1, 0:64], rz[:],
                                         start=True, stop=True)
                        o_ap = Q[p0:p0 + 64, ko, s * 512:(s + 1) * 512]
                        nc.scalar.activation(o_ap, pv[0:64, :], AF.Copy)
                        nc.vector.tensor_tensor(o_ap, o_ap, repz[:], ALU.mult)

            def out_proj(li, bias_sb):
                wts = []
                for half in range(2):
                    wt = wpool.tile([128, 3, D], BF16, tag="w")
                    nc.sync.dma_start(
                        wt[:], wo_r[ds(li, 1), :, 3 * half:3 * half + 3, :])
                    wts.append(wt)
                for m in range(KO):
                    pss = (psum(128, 512), psum(128, 512))
                    for kt in range(KO):
                        lhsT = wts[kt // 3][:, kt % 3, m * 128:(m + 1) * 128]
                        for c in range(2):
                            nc.tensor.matmul(
                                pss[c][:], lhsT, Q[:, kt, c * 512:(c + 1) * 512],
                                start=(kt == 0), stop=(kt == KO - 1))
                    for c in range(2):
                        cs = slice(c * 512, (c + 1) * 512)
                        nc.vector.tensor_tensor(R[:, m, cs], pss[c][:],
                                                X[:, m, cs], ALU.add)
                        nc.vector.tensor_scalar(
                            R[:, m, cs], R[:, m, cs],
                            bias_sb[:, 12 + m:13 + m], None, op0=ALU.add)

            def layer_norm():
                nc.scalar.activation(SQ[:], R[:], AF.Square)
                for c in range(2):
                    cs = slice(c * 512, (c + 1) * 512)
                    ps_s = psum(1, 512)
                    for kt in range(KO):
                        nc.tensor.matmul(ps_s[:], ones128r[:], R[:, kt, cs],
                                         start=(kt == 0), stop=(kt == KO - 1))
                    ps_q = psum(1, 512)
                    for kt in range(KO):
                        nc.tensor.matmul(ps_q[:], ones128b[:], SQ[:, kt, cs],
                                         start=(kt == 0), stop=(kt == KO - 1))
                    nc.vector.tensor_scalar_mul(mean_t[0:1, cs], ps_s[:], 1.0 / D)
                    nc.vector.tensor_scalar_mul(msq_t[0:1, cs], ps_q[:], 1.0 / D)
                nc.scalar.activation(b_t[:], mean_t[:], AF.Square)
                nc.vector.tensor_sub(msq_t[:], msq_t[:], b_t[:])
                nc.scalar.activation(b_t[:], msq_t[:], AF.Sqrt,
                                     bias=eps_t[:])
                nc.vector.reciprocal(a_t[:], b_t[:])
                nc.vector.tensor_mul(b_t[:], mean_t[:], a_t[:])
                for c in range(2):
                    cs = slice(c * 512, (c + 1) * 512)
                    pa = psum(128, 512)
                    nc.tensor.matmul(pa[:], ones1r[:], a_t[0:1, cs],
                                     start=True, stop=True)
                    pb = psum(128, 512)
                    nc.tensor.matmul(pb[:], ones1r[:], b_t[0:1, cs],
                                     start=True, stop=True)
                    pa_b = pa[:][:, None, :].to_broadcast((128, KO, 512))
                    pb_b = pb[:][:, None, :].to_broadcast((128, KO, 512))
                    nc.vector.tensor_tensor(X[:, :, cs], R[:, :, cs], pa_b,
                                            ALU.mult)
                    nc.vector.tensor_tensor(X[:, :, cs], X[:, :, cs], pb_b,
                                            ALU.subtract)
                    nc.scalar.activation(XB[:, :, cs], X[:, :, cs], AF.Copy)

            def ffn(li, bias_sb):
                for qc in range(2):
                    qs = slice(qc * 512, (qc + 1) * 512)
                    ys = [psum(128, 512) for _ in range(KO)]
                    for ft in range(FO):
                        if ft % 2 == 0:
                            # 2-ft granularity keeps bf16 DRAM runs at 512B
                            w1t2 = w1p.tile([128, KO, 256], BF16, tag="w1")
                            nc.sync.dma_start(
                                w1t2[:],
                                w1_r[ds(li, 1), :, :, ft * 128:(ft + 2) * 128])
                        w1t = w1t2[:, :, (ft % 2) * 128:(ft % 2) * 128 + 128]
                        w2t = w2p.tile([128, D], BF16, tag="w2")
                        nc.sync.dma_start(
                            w2t[:], w2_d[ds(li, 1), ft * 128:(ft + 1) * 128, :])
                        ph = psum(128, 512)
                        for kt in range(KO):
                            nc.tensor.matmul(ph[:], w1t[:, kt, :], XB[:, kt, qs],
                                             start=(kt == 0), stop=(kt == KO - 1))
                        h_sb = hpool.tile([128, 512], BF16, tag="h")
                        nc.scalar.activation(h_sb[:], ph[:], AF.Relu,
                                             bias=bias_sb[:, 24 + ft:25 + ft])
                        for j in range(KO):
                            nc.tensor.matmul(
                                ys[j][:], w2t[:, j * 128:(j + 1) * 128], h_sb[:],
                                start=(ft == 0), stop=(ft == FO - 1),
                                skip_group_check=True)
                    for j in range(KO):
                        nc.vector.tensor_tensor(R[:, j, qs], ys[j][:], X[:, j, qs],
                                                ALU.add)
                        nc.vector.tensor_scalar(
                            R[:, j, qs], R[:, j, qs],
                            bias_sb[:, 18 + j:19 + j], None, op0=ALU.add)

            with tc.For_i(0, n_reps) as _rep:
                with tc.For_i(0, L) as li:
                    bias_sb = biasp.tile([128, NBIAS], F32, tag="b")
                    nc.sync.dma_start(bias_sb[:], bias_d[ds(li, 1), :, :])
                    qk_proj(li, wq_r, 0, Q, bias_sb)
                    qk_proj(li, wk_r, 6, K, bias_sb)
                    v_proj(li, bias_sb)
                    attention()
                    out_proj(li, bias_sb)
                    layer_norm()
                    ffn(li, bias_sb)
                    layer_norm()

            nc.sync.dma_start(xout_d[:], X[:].bitcast(F32))

    nc.compile()
    return nc


_NC_CACHE = {}


def _get_nc(n_reps=1):
    if n_reps not in _NC_CACHE:
        _NC_CACHE[n_reps] = _build(n_reps)
    return _NC_CACHE[n_reps]


def _to_bf16(a):
    # round-to-nearest-even fp32 -> bf16, stored as the raw uint16 view
    # reinterpreted through ml_dtypes for numpy transport
    import ml_dtypes
    return np.asarray(a, dtype=np.float32).astype(ml_dtypes.bfloat16)


def _prep_in_maps(inputs):
    tokens = np.asarray(inputs["tokens"])
    segments = np.asarray(inputs["segments"])
    valid_lens = np.asarray(inputs["valid_lens"])
    tok_emb = np.asarray(inputs["tok_emb"], dtype=np.float32)
    seg_emb = np.asarray(inputs["seg_emb"], dtype=np.float32)
    pos_emb = np.asarray(inputs["pos_emb"], dtype=np.float32)
    Wq = np.ascontiguousarray(np.asarray(inputs["Wq"], dtype=np.float32))
    Wk = np.ascontiguousarray(np.asarray(inputs["Wk"], dtype=np.float32))
    Wv = np.ascontiguousarray(np.asarray(inputs["Wv"], dtype=np.float32))
    Wo = np.ascontiguousarray(np.asarray(inputs["Wo"], dtype=np.float32))
    W1 = np.ascontiguousarray(np.asarray(inputs["W1"], dtype=np.float32))
    W2 = np.ascontiguousarray(np.asarray(inputs["W2"], dtype=np.float32))

    # host-side embedding + positional/segment add
    X0 = tok_emb[tokens] + seg_emb[segments] + pos_emb[0, :S][None, :, :]
    X0 = X0.astype(np.float32)

    # bias pack: bq | bk | bo_eff = bv @ Wo + bo | b2 | b1
    bo_eff = np.einsum("ld,ldm->lm", np.asarray(inputs["bv"], np.float32), Wo) \
        + np.asarray(inputs["bo"], np.float32)
    bias_pack = np.empty((L, 128, NBIAS), np.float32)
    for name, lo in (("bq", 0), ("bk", 6)):
        arr = np.asarray(inputs[name], np.float32)
        bias_pack[:, :, lo:lo + 6] = arr.reshape(L, 6, 128).transpose(0, 2, 1)
    bias_pack[:, :, 12:18] = bo_eff.reshape(L, 6, 128).transpose(0, 2, 1)
    bias_pack[:, :, 18:24] = np.asarray(inputs["b2"], np.float32) \
        .reshape(L, 6, 128).transpose(0, 2, 1)
    bias_pack[:, :, 24:48] = np.asarray(inputs["b1"], np.float32) \
        .reshape(L, 24, 128).transpose(0, 2, 1)
    bias_pack = np.ascontiguousarray(bias_pack)

    shared = dict(WQ=_to_bf16(Wq), WK=_to_bf16(Wk), WV=_to_bf16(Wv),
                  WO=_to_bf16(Wo), W1=_to_bf16(W1), W2=_to_bf16(W2),
                  BIAS=bias_pack)

    kpos = np.arange(S)
    in_maps = []
    for c in range(N_CORES):
        xc = X0[SB * c:SB * (c + 1)].reshape(T, D).T            # [768, 1024]
        xc = np.ascontiguousarray(
            xc.reshape(KO, 128, T).transpose(1, 0, 2))          # [128, 6, 1024]
        mc = np.zeros((128, SB * 4), np.float32)
        for s in range(SB):
            vl = valid_lens[SB * c + s]
            m = np.where(kpos >= vl, np.float32(NEG), np.float32(0.0))
            mc[:, s * 4:(s + 1) * 4] = m.reshape(4, 128).T
        in_maps.append(dict(x0=xc, mask=np.ascontiguousarray(mc), **shared))
    return in_maps


def _unshard(results):
    out = np.empty((B, S, D), np.float32)
    for c in range(N_CORES):
        xt = results[c]["xout"]                                  # [128, 6, 1024]
        xt = xt.transpose(1, 0, 2).reshape(D, T)                 # [768, 1024]
        out[SB * c:SB * (c + 1)] = xt.T.reshape(SB, S, D)
    return out


def kernel(**inputs):
    nc = _get_nc(1)
    in_maps = _prep_in_maps(inputs)
    res = run_bass_kernel_spmd(nc, in_maps, list(range(N_CORES)), trace=False)
    return _unshard(res.results)


if __name__ == "__main__":
    import time
    t0 = time.time()
    nc = _build(int(os.environ.get("NREPS", "1")))
    print("build+compile time:", time.time() - t0)
